# revision 1
# baseline (speedup 1.0000x reference)
"""Trainium2 Bass kernel for DebiasNtXentLoss (B=4096, D=128, 8 NeuronCores).

Symmetry-exploiting data-parallel decomposition: each core exps 5M instead
of 8.4M elements.

sim is symmetric, so block-pair (a, b) only needs computing once.  With znt
rotated by c*1024 per core, core c computes row-block c against col-blocks
c..c+4 (local cols 0..5120):
  d=0   diagonal block, row sums only (full 1024x1024, no mirror needed)
  d=1-3 full-weight slabs: row sums for my rows + column sums (the mirror
        row-sum contribution for blocks c+1..c+3, shipped to the host)
  d=4   the antipodal pair {c, c+4} is computed by BOTH core c and c+4, so
        its exp values are halved on the fly via exp(2x + ln(1/2)) — the
        ACT bias input — making row+col contributions sum to exactly 1x.
Column sums are ones^T @ etile PE matmuls accumulated over the 8 row tiles
in PSUM.  The host adds the 8 cores' row/col partials into the full
rowsum[8192], computes pos/self from zn (0.05% of FLOPs), and finishes the
scalar loss.
"""

import numpy as np

import concourse.bacc as bacc
import concourse.bass as bass
import concourse.mybir as mybir
import concourse.tile as tile
from concourse.bass_utils import run_bass_kernel_spmd

B = 4096
D = 128
N = 2 * B
NCORES = 8
RPC = N // NCORES      # 1024
MYT = RPC // 128       # 8 row tiles
NCOL = 5 * RPC         # 5120 cols of znt shipped per core

TEMPERATURE = 0.5
RHO = 0.1
N_NEG = N - 2
INV_T = 1.0 / TEMPERATURE
LN_HALF = float(np.log(0.5))
FLOOR = float(np.float32(N_NEG) * np.float32(np.exp(-1.0 / TEMPERATURE)))

F32 = mybir.dt.float32
BF16 = mybir.dt.bfloat16
AF = mybir.ActivationFunctionType
ALU = mybir.AluOpType
AX = mybir.AxisListType

_CACHE = {}


def _build():
    nc = bacc.Bacc("TRN2", target_bir_lowering=False, debug=False)
    znt_dram = nc.dram_tensor("znt", [128, NCOL], BF16, kind="ExternalInput")
    rs_dram = nc.dram_tensor("rs", [128, MYT], F32, kind="ExternalOutput")
    cols_dram = nc.dram_tensor("cols", [8, 512], F32, kind="ExternalOutput")

    with tile.TileContext(nc) as tc:
        with (
            tc.tile_pool(name="big", bufs=1) as big,
            tc.tile_pool(name="small", bufs=1) as small,
            tc.tile_pool(name="et", bufs=3) as etp,
            tc.tile_pool(name="psum", bufs=2, space=bass.MemorySpace.PSUM) as pp,
        ):
            # warmup: get the exp table loaded during the DMA phase
            w = small.tile([128, 1], F32)
            nc.vector.memset(w[:], 0.0)
            w2 = small.tile([128, 1], F32)
            nc.scalar.activation(w2[:], w[:], AF.Exp)

            ones = small.tile([128, 128], BF16)
            nc.vector.memset(ones[:], 1.0)

            znt = big.tile([128, NCOL], BF16)
            # retained exp tiles (needed later for the column-sum matmuls)
            et_w = big.tile([128, MYT, 2048], BF16)   # d=1,2  (cols 1024..3072)
            et_34 = big.tile([128, MYT, 2, 1024], BF16)  # d3 | d4 per m
            acc_w = small.tile([128, MYT], F32)
            acc_d = small.tile([128, MYT], F32)
            acc_34 = small.tile([128, MYT, 2], F32)
            cs_sb = big.tile([128, 8, 512], F32)

            # input DMA: interleave the two queues, first chunks first
            for h in range(5):
                eng = nc.sync if h % 2 == 0 else nc.gpsimd
                eng.dma_start(
                    znt[:, h * 1024 : (h + 1) * 1024],
                    znt_dram.ap()[:, h * 1024 : (h + 1) * 1024],
                )

            wt = pp.tile([128, 2048], F32, tag="mm")
            for _ in range(40):
                nc.tensor.matmul(wt[:, 0:128], ones[:], ones[:],
                                 start=True, stop=True)

            def slab_mms(pt, m, c0, ncols):
                """ncols matmuls of 512 for row tile m at col offset c0."""
                for j in range(ncols):
                    nc.tensor.matmul(
                        pt[:, j * 512 : (j + 1) * 512],
                        znt[:, m * 128 : (m + 1) * 128],
                        znt[:, c0 + j * 512 : c0 + (j + 1) * 512],
                        start=True,
                        stop=True,
                    )

            # ---- diagonal d=0 (cols 0..1024): bare exp + DVE reduce ----
            for u in range(4):
                pt = pp.tile([128, 2048], F32, tag="mm")
                for i in range(2):
                    slab_mms(pt[:, i * 1024 : (i + 1) * 1024], 2 * u + i, 0, 2)
                et = etp.tile([128, 2048], BF16, tag="etd")
                nc.scalar.activation(et[:], pt[:], AF.Exp, scale=INV_T)
                nc.vector.reduce_sum(
                    acc_d[:, 2 * u : 2 * u + 2],
                    et[:].rearrange("p (i x) -> p i x", i=2),
                    axis=AX.X,
                )

            # ---- wide slab d=1,2 (cols 1024..3072): per-m ACT accum ----
            for m in range(MYT):
                pt = pp.tile([128, 2048], F32, tag="mm")
                slab_mms(pt, m, 1024, 4)
                nc.scalar.activation(
                    et_w[:, m, :], pt[:], AF.Exp, scale=INV_T,
                    accum_out=acc_w[:, m : m + 1],
                )

            # ---- column sums: ones^T @ etile over a row-tile range ----
            def cs_chunk(k, rhs_of_m, m_lo=0, m_hi=MYT):
                cp = pp.tile([128, 2048], F32, tag="mm")
                for m in range(m_lo, m_hi):
                    nc.tensor.matmul(
                        cp[:, 0:512],
                        ones[:],
                        rhs_of_m(m),
                        start=(m == m_lo),
                        stop=(m == m_hi - 1),
                    )
                nc.vector.tensor_copy(cs_sb[:, k, :], cp[:, 0:512])

            def d34_unit(m):
                # d=3 and d=4 columns for one row tile: 4 matmuls sharing
                # one lhsT (full weight — d4's mirror is the partner core's
                # own d4 row sums, nothing to halve or ship)
                pt = pp.tile([128, 2048], F32, tag="mm")
                slab_mms(pt[:, 0:1024], m, 3072, 2)
                slab_mms(pt[:, 1024:2048], m, 4096, 2)
                nc.scalar.activation(
                    et_34[:, m],
                    pt[:].rearrange("p (i x) -> p i x", i=2),
                    AF.Exp,
                    scale=INV_T,
                )
                nc.vector.reduce_sum(acc_34[:, m, :], et_34[:, m], axis=AX.X)

            csw = lambda k: cs_chunk(k, lambda m, k=k: et_w[:, m, k * 512 : (k + 1) * 512])

            def cs3(slot, k, m_lo, m_hi):
                cs_chunk(slot, lambda m, k=k: et_34[:, m, 0, k * 512 : (k + 1) * 512],
                         m_lo, m_hi)

            # interleave: cs chunks ride between d34 units so the PE keeps
            # feeding ACT with fresh slab PSUM while summing columns.
            # cs3 splits into two 4-tile halves (summed on the host) so each
            # half only needs the d34 units already finished.
            d34_unit(0); csw(0)
            d34_unit(1); csw(1)
            d34_unit(2); csw(2)
            d34_unit(3); csw(3)
            d34_unit(4); cs3(4, 0, 0, 4)
            d34_unit(5); cs3(5, 1, 0, 4)
            d34_unit(6)
            d34_unit(7)
            cs3(6, 0, 4, 8); cs3(7, 1, 4, 8)

            # ---- assemble row-sum partial and ship everything out ----
            rs = small.tile([128, MYT], F32)
            acc_34r = small.tile([128, MYT], F32)
            nc.vector.reduce_sum(acc_34r[:], acc_34[:], axis=AX.X)
            nc.vector.tensor_add(rs[:], acc_w[:], acc_d[:])
            nc.vector.tensor_add(rs[:], rs[:], acc_34r[:])
            nc.gpsimd.dma_start(rs_dram.ap(), rs[:])
            nc.gpsimd.dma_start(cols_dram.ap(), cs_sb[0:1, :, :])

    nc.compile()
    return nc


def _get_nc():
    if "nc" not in _CACHE:
        _CACHE["nc"] = _build()
    return _CACHE["nc"]


def _prep_inputs(z_i, z_j):
    import ml_dtypes

    z = np.concatenate(
        [np.asarray(z_i, np.float32), np.asarray(z_j, np.float32)], axis=0
    )
    zn = z / np.maximum(
        np.sqrt((z * z).sum(axis=1, keepdims=True, dtype=np.float32)), 1e-8
    ).astype(np.float32)
    znt = np.ascontiguousarray(zn.T).astype(ml_dtypes.bfloat16)  # [128, 8192]
    in_maps = []
    for c in range(NCORES):
        znt_c = np.roll(znt, -c * RPC, axis=1)[:, :NCOL]
        in_maps.append({"znt": np.ascontiguousarray(znt_c)})
    return in_maps, zn


def kernel(z_i, z_j, _want_results=False, **run_kwargs):
    nc = _get_nc()
    in_maps, zn = _prep_inputs(z_i, z_j)
    out = run_bass_kernel_spmd(
        nc, in_maps, core_ids=list(range(NCORES)), **run_kwargs
    )
    rowsum = np.zeros(N, dtype=np.float64)
    for c in range(NCORES):
        r = out.results[c]
        # rs[p, m] = partial rowsum of global row c*1024 + m*128 + p
        rowsum[c * RPC : (c + 1) * RPC] += r["rs"].T.reshape(-1).astype(np.float64)
        # cols[k] covers global cols (c+1)*1024 + k*512 .. +512 (mod N)
        for j in range(8):
            kk = j if j < 4 else 4 + (j - 4) % 2
            g0 = (c * RPC + RPC + kk * 512) % N
            rowsum[g0 : g0 + 512] += r["cols"][j].astype(np.float64)

    zn64 = zn.astype(np.float64)
    pos = np.exp(INV_T * np.sum(zn64 * np.roll(zn64, -B, axis=0), axis=1))
    slf = np.exp(INV_T * np.sum(zn64 * zn64, axis=1))
    neg = rowsum - slf - pos
    ng = (-RHO * N_NEG * pos + neg) / (1.0 - RHO)
    ng = np.maximum(ng, N_NEG * np.exp(-1.0 / TEMPERATURE))
    losses = np.log(pos + ng) - np.log(pos)
    loss = np.float32(losses.mean())
    if _want_results:
        return loss, out
    return loss

